# revision 12
# baseline (speedup 1.0000x reference)
"""Trainium2 Bass kernel for DecoderSplattingCUDA (Gaussian splatting renderer).

Contract: kernel(**inputs) takes the FULL unsharded inputs (as produced by
setup_inputs) and returns the FULL [1, 2, 3, 64, 64] float32 output.

Sharding: 8 cores = 2 cameras x 4 pixel row-bands (16 rows of 64 px each).
Per-core the host culls the depth-sorted gaussians to those whose alpha can
reach 1/255 inside the band (safe ellipse bound), padded to G_DEV (1024, with
a 2048 fallback program if any core overflows).

Device pipeline per core (gaussians on partitions in blocks of 128, pixels on
the free axis in chunks of 512):
  1. pb = power + ln(op) via K=15 fp16 matmul against a hi/lo-split pixel
     basis (exact to ~1e-3 despite fp16 operands; PSUM accumulates fp32).
  2. alpha = Exp(pb) -> fp16; cutoff alpha<1/255 and clamp at 0.99 via
     is_ge + fused (min, mult) scalar_tensor_tensor (DVE, fp16 2x mode).
  3. L = Ln(1 - alpha) -> fp16 (ACT, scale=-1 bias=1).
  4. Inclusive depth scan S = cumsum(L): upper-triangular fp16 matmul per
     block + per-block carry matmul of previous block totals (PE, fp32 PSUM).
  5. T = Exp(S) -> fp16 = inclusive transmittance.
  6. color = c0 + sum_g T_g * D_g  (Abel summation: D_g = rgb_{g+1} - rgb_g,
     D_last = bg - rgb_last) via [128,3] fp16 matmuls into fp32 PSUM.
"""

import os
import sys

import numpy as np

for _p in ("/opt/trn_rl_repo", "/root/.axon_site/_ro/trn_rl_repo"):
    if os.path.isdir(_p) and _p not in sys.path:
        sys.path.insert(0, _p)
        break

import concourse.bass as bass  # noqa: E402
import concourse.mybir as mybir  # noqa: E402
from concourse.mybir import AluOpType  # noqa: E402
from concourse.tile import TileContext  # noqa: E402
from concourse.bass_utils import run_bass_kernel_spmd  # noqa: E402

# ---------------------------------------------------------------------------
# Workaround: this walrus build only accepts a single sync-wait per
# instruction. Post-pass: hoist all-but-one wait of any multi-wait
# instruction into preceding same-engine NoOps (semantics preserved: the
# engine blocks on each wait in sequence before executing the instruction).
# ---------------------------------------------------------------------------


def _split_multi_waits(nc):
    n = 0
    for f in nc.m.functions:
        for bb in f.blocks:
            new = []
            changed = False
            for ins in bb.instructions:
                si = ins.sync_info
                if si is not None and len(si.on_wait) > 1:
                    changed = True
                    waits = list(si.on_wait)
                    for w in waits[:-1]:
                        n += 1
                        nop = mybir.InstNoOp(name=f"I-wsplit-{n}", ins=[],
                                             outs=[])
                        nop.engine = ins.engine
                        nop.sync_info = mybir.SyncInfo(on_wait=[w],
                                                       on_update=[])
                        new.append(nop)
                    ins.sync_info = mybir.SyncInfo(
                        on_wait=[waits[-1]], on_update=list(si.on_update))
                new.append(ins)
            if changed:
                bb.instructions = new
    return n


# ---------------------------------------------------------------------------
# Problem constants (hardcoded per spec)
# ---------------------------------------------------------------------------
SH_C0 = 0.28209479177387814
NEAR, FAR = 0.1, 1000.0
H = W = 64
G = 2048          # total gaussians (2 context views x 32 x 32)
NCAM = 2
NBAND = 4         # row bands per camera
BAND_ROWS = H // NBAND   # 16
P = BAND_ROWS * W        # 1024 pixels per core
NCHUNK = 2
CH = P // NCHUNK         # 512
F32 = mybir.dt.float32
F16 = mybir.dt.float16
KPOW = 15                # hi/lo-split power matmul contraction size
MASK_CONST = -60000.0    # fp16-representable "minus infinity" for pb
ALPHA_MIN = np.float32(1.0) / np.float32(255.0)
ALPHA_MAX = np.float32(0.99)

_PROGRAMS = {}


def _build_program(g_dev):
    nb = g_dev // 128
    nc = bass.Bass(target_bir_lowering=False)

    coeff = nc.declare_dram_parameter("coeff", [KPOW, g_dev], F16,
                                      isOutput=False)
    pixb = nc.declare_dram_parameter("pixb", [KPOW, P], F16, isOutput=False)
    dpack = nc.declare_dram_parameter("dpack", [128, nb * 3], F16,
                                      isOutput=False)
    c0col = nc.declare_dram_parameter("c0col", [3, 1], F32, isOutput=False)
    trimat = nc.declare_dram_parameter("trimat", [128, 128], F16,
                                       isOutput=False)
    colsel = nc.declare_dram_parameter("colsel", [128, nb * nb], F16,
                                       isOutput=False)
    carrysel = nc.declare_dram_parameter("carrysel", [nb, g_dev], F16,
                                         isOutput=False)
    outc = nc.declare_dram_parameter("outc", [3, P], F32, isOutput=True)

    with TileContext(nc) as tc:
        with (
            tc.tile_pool(name="consts", bufs=1) as consts,
            tc.tile_pool(name="lbuf", bufs=1) as lbuf,
            tc.tile_pool(name="apool", bufs=3) as apool,
            tc.tile_pool(name="a2pool", bufs=3) as a2pool,
            tc.tile_pool(name="mpool", bufs=3) as mpool,
            tc.tile_pool(name="tpool", bufs=3) as tpool,
            tc.tile_pool(name="totsb", bufs=1) as totsb,
            tc.tile_pool(name="outsb", bufs=1) as outsb,
            tc.tile_pool(name="ps_pbs", bufs=2, space="PSUM") as ps_pbs,
            tc.tile_pool(name="ps_tot", bufs=1, space="PSUM") as ps_tot,
            tc.tile_pool(name="ps_col", bufs=1, space="PSUM") as ps_col,
        ):
            # dummy-matmul operand tiles for HAM warm-up bursts (the PE clock
            # gate needs ~3.4us of contiguous matmul activity to open)
            w_lhs = consts.tile([128, 128], F16, tag="w_lhs")
            nc.vector.memset(w_lhs, 0.0)
            w_rhs = consts.tile([128, CH], F16, tag="w_rhs")
            nc.vector.memset(w_rhs, 0.0)

            s_coeff = consts.tile([KPOW, g_dev], F16, tag="coeff")
            nc.sync.dma_start(out=s_coeff, in_=coeff[:])
            s_pixb = consts.tile([KPOW, P], F16, tag="pixb")
            nc.sync.dma_start(out=s_pixb, in_=pixb[:])
            s_dpack = consts.tile([128, nb * 3], F16, tag="dpack")
            nc.sync.dma_start(out=s_dpack, in_=dpack[:])
            s_c0 = consts.tile([3, 1], F32, tag="c0col")
            nc.sync.dma_start(out=s_c0, in_=c0col[:])
            s_tri = consts.tile([128, 128], F16, tag="trimat")
            nc.sync.dma_start(out=s_tri, in_=trimat[:])
            s_colsel = consts.tile([128, nb * nb], F16, tag="colsel")
            nc.sync.dma_start(out=s_colsel, in_=colsel[:])
            s_carry = consts.tile([nb, g_dev], F16, tag="carrysel")
            nc.sync.dma_start(out=s_carry, in_=carrysel[:])

            # big L buffer: [128, nb, P] fp16
            l_all = lbuf.tile([128, nb, P], F16, tag="l_all")
            # psum accumulators alive through the pass
            tot_ps = ps_tot.tile([nb, P], F32, tag="tot")
            col_ps = ps_col.tile([3, P], F32, tag="col")

            # ---- Pass A: alpha + L per gaussian block ----
            for b in range(nb):
                pb_ps = ps_pbs.tile([128, P], F32, tag="pbs")
                for ch in range(NCHUNK):
                    nc.tensor.matmul(
                        pb_ps[:, ch * CH:(ch + 1) * CH],
                        s_coeff[:, b * 128:(b + 1) * 128],
                        s_pixb[:, ch * CH:(ch + 1) * CH],
                        start=True,
                        stop=True,
                    )
                a_t = apool.tile([128, P], F16, tag="a")
                nc.scalar.activation(
                    a_t, pb_ps, mybir.ActivationFunctionType.Exp,
                )
                # m = (alpha >= 1/255) * 0.99  in {0, 0.99}
                m_t = mpool.tile([128, P], F16, tag="m")
                nc.vector.tensor_scalar(
                    out=m_t, in0=a_t, scalar1=float(ALPHA_MIN),
                    scalar2=float(ALPHA_MAX),
                    op0=AluOpType.is_ge, op1=AluOpType.mult,
                )
                # masked+clamped alpha = min(alpha, m)
                a2_t = a2pool.tile([128, P], F16, tag="a2")
                nc.vector.tensor_tensor(
                    out=a2_t, in0=a_t, in1=m_t, op=AluOpType.min,
                )
                nc.scalar.activation(
                    l_all[:, b, :],
                    a2_t,
                    mybir.ActivationFunctionType.Ln,
                    bias=1.0,
                    scale=-1.0,
                )
                for ch in range(NCHUNK):
                    nc.tensor.matmul(
                        tot_ps[:, ch * CH:(ch + 1) * CH],
                        s_colsel[:, b * nb:(b + 1) * nb],
                        l_all[:, b, ch * CH:(ch + 1) * CH],
                        start=(b == 0),
                        stop=(b == nb - 1),
                    )

            # totals -> SBUF (fp16 cast; |totals| <= ~50 so fp16 rel err ok)
            tot_sb = totsb.tile([nb, P], F16, tag="totsb")
            nc.vector.tensor_copy(tot_sb, tot_ps)

            # keep the PE HAM warm across the pass A -> pass B transition
            # (the totals cast leaves the PE idle long enough to re-throttle)
            for _ in range(9):
                w_ps = ps_pbs.tile([128, CH], F32, tag="pbs")
                nc.tensor.matmul(w_ps, w_lhs, w_rhs, start=True, stop=True)

            # ---- Pass B: scan + carry + T + color ----
            # same-lhsT matmuls adjacent (tri, tri, carry, carry) to cut
            # LDWEIGHTS churn
            for b in range(nb):
                s_ps = ps_pbs.tile([128, P], F32, tag="pbs")
                for ch in range(NCHUNK):
                    nc.tensor.matmul(
                        s_ps[:, ch * CH:(ch + 1) * CH],
                        s_tri,
                        l_all[:, b, ch * CH:(ch + 1) * CH],
                        start=True,
                        stop=(b == 0),
                    )
                if b > 0:
                    for ch in range(NCHUNK):
                        nc.tensor.matmul(
                            s_ps[:, ch * CH:(ch + 1) * CH],
                            s_carry[:, b * 128:(b + 1) * 128],
                            tot_sb[:, ch * CH:(ch + 1) * CH],
                            start=False,
                            stop=True,
                        )
                t_t = tpool.tile([128, P], F16, tag="t")
                nc.scalar.activation(
                    t_t, s_ps, mybir.ActivationFunctionType.Exp,
                )
                for ch in range(NCHUNK):
                    nc.tensor.matmul(
                        col_ps[:, ch * CH:(ch + 1) * CH],
                        s_dpack[:, b * 3:(b + 1) * 3],
                        t_t[:, ch * CH:(ch + 1) * CH],
                        start=(b == 0),
                        stop=(b == nb - 1),
                    )

            # out = color + c0 (per-partition scalar add folds the Abel
            # constant term into the output copy)
            out_sb = outsb.tile([3, P], F32, tag="outsb")
            nc.vector.tensor_scalar(
                out=out_sb, in0=col_ps, scalar1=s_c0[:, 0:1], scalar2=None,
                op0=AluOpType.add,
            )
            nc.sync.dma_start(out=outc[:], in_=out_sb)

    _split_multi_waits(nc)
    return nc


def _get_program(g_dev):
    if g_dev not in _PROGRAMS:
        _PROGRAMS[g_dev] = _build_program(g_dev)
    return _PROGRAMS[g_dev]


# ---------------------------------------------------------------------------
# Host-side geometry / layout prep
# ---------------------------------------------------------------------------


def _hi_lo(x):
    hi = x.astype(np.float16)
    lo = (x - hi.astype(np.float64)).astype(np.float16)
    return hi, lo


def _project(base_pose, target_pose, intrinsics, means, cov, sh, op):
    """Per-camera projection. Returns per-camera dict of sorted per-gaussian
    quantities (float64 except z which mirrors the reference's fp32)."""
    f32 = np.float32
    inv_base = np.linalg.inv(base_pose.astype(f32))            # [1,4,4]
    extr = np.einsum("bij,bvjk->bvik", inv_base,
                     target_pose.astype(f32)).reshape(NCAM, 4, 4)
    view = np.linalg.inv(extr.astype(f32))
    R = view[:, :3, :3].astype(f32)
    t = view[:, :3, 3].astype(f32)

    K = intrinsics.reshape(NCAM, 3, 3).astype(np.float64)
    cams = []
    for c in range(NCAM):
        p = (means.astype(f32) @ R[c].T.astype(f32) + t[c]).astype(f32)
        z = p[:, 2]                                            # fp32 ref-like
        zc = np.maximum(z, f32(1e-6)).astype(np.float64)
        x = p[:, 0].astype(np.float64)
        y = p[:, 1].astype(np.float64)
        fx, fy = K[c, 0, 0], K[c, 1, 1]
        cx, cy = K[c, 0, 2], K[c, 1, 2]
        u = fx * x / zc + cx
        v = fy * y / zc + cy
        R64 = R[c].astype(np.float64)
        cov_cam = np.einsum("ij,gjk,lk->gil", R64, cov.astype(np.float64),
                            R64)
        w1 = fx / zc
        w2 = -fx * x / zc ** 2
        w3 = fy / zc
        w4 = -fy * y / zc ** 2
        c00, c01, c02 = cov_cam[:, 0, 0], cov_cam[:, 0, 1], cov_cam[:, 0, 2]
        c11, c12, c22 = cov_cam[:, 1, 1], cov_cam[:, 1, 2], cov_cam[:, 2, 2]
        a2d = w1 * w1 * c00 + 2.0 * w1 * w2 * c02 + w2 * w2 * c22 + 0.3
        b2d = (w1 * w3 * c01 + w1 * w4 * c02 + w2 * w3 * c12
               + w2 * w4 * c22)
        d2d = w3 * w3 * c11 + 2.0 * w3 * w4 * c12 + w4 * w4 * c22 + 0.3
        det = a2d * d2d - b2d * b2d
        inv_det = 1.0 / det
        ca = d2d * inv_det
        cb = -b2d * inv_det
        cc = a2d * inv_det
        lam_max_cov = 0.5 * ((a2d + d2d)
                             + np.sqrt((a2d - d2d) ** 2 + 4 * b2d * b2d))
        valid = (z > NEAR) & (z < FAR) & (det > 0) & (op > ALPHA_MIN)
        rgb = np.maximum(SH_C0 * sh[:, :, 0].astype(np.float64) + 0.5, 0.0)
        order = np.argsort(z, kind="stable")
        cams.append(dict(
            u=u[order] - W / 2.0, v=v[order] - H / 2.0,
            ca=ca[order], cb=cb[order], cc=cc[order],
            lam=lam_max_cov[order], valid=valid[order],
            op=np.asarray(op, np.float64)[order], rgb=rgb[order],
        ))
    return cams


def _band_keep(cam, band):
    """Safe cull: keep gaussians whose alpha can reach 1/255 in the band."""
    px_lo, px_hi = 0.5 - W / 2.0, (W - 0.5) - W / 2.0
    py_lo = band * BAND_ROWS + 0.5 - H / 2.0
    py_hi = band * BAND_ROWS + BAND_ROWS - 0.5 - H / 2.0
    dx = np.maximum(np.maximum(px_lo - cam["u"], cam["u"] - px_hi), 0.0)
    dy = np.maximum(np.maximum(py_lo - cam["v"], cam["v"] - py_hi), 0.0)
    # alpha >= 1/255 requires quadform <= 2*(ln op - ln(1/255));
    # quadform >= dist^2 / lam_max(cov2d). 10% + 1px safety inflation.
    budget = 2.0 * (np.log(np.maximum(cam["op"], 1e-12))
                    - np.log(float(ALPHA_MIN)))
    reach2 = np.maximum(budget, 0.0) * cam["lam"]
    reach = np.sqrt(np.maximum(reach2, 0.0)) * 1.05 + 1.0
    return cam["valid"] & (dx * dx + dy * dy <= reach * reach)


def _pack_core(cam, band, g_dev, bg):
    """Build the per-core device inputs for one (camera, band)."""
    keep = _band_keep(cam, band)
    idx = np.nonzero(keep)[0]            # preserves depth order
    n = len(idx)
    assert n <= g_dev
    nb = g_dev // 128

    def sel(a):
        return a[idx]

    u, v = sel(cam["u"]), sel(cam["v"])
    ca, cb, cc = sel(cam["ca"]), sel(cam["cb"]), sel(cam["cc"])
    opk = sel(cam["op"])
    A = -0.5 * (ca * u * u + cc * v * v) - cb * u * v
    const = A + np.log(np.maximum(opk, 1e-12))
    B = ca * u + cb * v
    Cc = cc * v + cb * u
    D = -0.5 * ca
    E = -0.5 * cc
    F = -cb

    def pad(a):
        out = np.zeros(g_dev, np.float64)
        out[:n] = a
        return out

    A6 = [pad(D), pad(E), pad(F), pad(B), pad(Cc)]
    constp = np.full(g_dev, MASK_CONST, np.float64)
    constp[:n] = np.clip(const, MASK_CONST, 0.0)
    # fp16 hi/lo split of the 6 coefficient rows -> 15 rows matching
    # the duplicated pixel basis rows (see _pix_basis15).
    rows = []
    for cf in A6[:3]:                      # quadratic coeffs: hi, hi, lo
        hi, lo = _hi_lo(cf)
        rows += [hi, hi, lo]
    # reorder: we emitted [Dhi,Dhi,Dlo, Ehi,Ehi,Elo, Fhi,Fhi,Flo]
    for cf in A6[3:]:                      # linear coeffs: hi, lo
        hi, lo = _hi_lo(cf)
        rows += [hi, lo]
    khi, klo = _hi_lo(constp)
    rows += [khi, klo]
    coeff15 = np.stack(rows).astype(np.float16)     # [15, g_dev]

    rgb = sel(cam["rgb"])                           # [n, 3]
    if n == 0:
        c0 = bg.astype(np.float64)
        Dr = np.zeros((g_dev, 3), np.float64)
    else:
        c0 = rgb[0]
        rgb_p = np.concatenate(
            [rgb, np.repeat(rgb[-1:], g_dev - n, axis=0)], 0)
        Dr = np.empty((g_dev, 3), np.float64)
        Dr[:-1] = rgb_p[1:] - rgb_p[:-1]
        Dr[-1] = bg.astype(np.float64) - rgb_p[-1]
    dpack = Dr.reshape(nb, 128, 3).transpose(1, 0, 2).reshape(128, nb * 3)
    return (coeff15, dpack.astype(np.float16),
            c0.astype(np.float32).reshape(3, 1), n)


def _shared_consts(g_dev):
    f16 = np.float16
    nb = g_dev // 128
    trimat = (np.arange(128)[:, None] <= np.arange(128)[None, :]).astype(f16)
    colsel = np.zeros((128, nb, nb), f16)
    for b in range(nb):
        colsel[:, b, b] = 1.0
    colsel = colsel.reshape(128, nb * nb)
    carrysel = np.zeros((nb, nb, 128), f16)
    for b in range(nb):
        carrysel[:b, b, :] = 1.0
    carrysel = carrysel.reshape(nb, g_dev)
    return trimat, colsel, carrysel


def _pix_basis15(band):
    px = (np.arange(W, dtype=np.float64) + 0.5) - W / 2.0
    py = (np.arange(BAND_ROWS, dtype=np.float64)
          + band * BAND_ROWS + 0.5) - H / 2.0
    gy, gx = np.meshgrid(py, px, indexing="ij")
    gx = gx.reshape(-1)
    gy = gy.reshape(-1)
    one = np.ones_like(gx)
    q = {}
    for name, val in (("xx", gx * gx), ("yy", gy * gy), ("xy", gx * gy)):
        q[name] = _hi_lo(val)
    # rows match coeff15: [Dhi*xxhi, Dhi*xxlo, Dlo*xxhi] etc.
    rows = [q["xx"][0], q["xx"][1], q["xx"][0],
            q["yy"][0], q["yy"][1], q["yy"][0],
            q["xy"][0], q["xy"][1], q["xy"][0],
            gx, gx, gy, gy, one, one]
    return np.stack([np.asarray(r, np.float64) for r in rows]).astype(np.float16)


def kernel(base_pose, target_pose, intrinsics, means1, covariances1, sh1,
           opacities1, means2, covariances2, sh2, opacities2,
           background_color, h_out, w_out):
    assert int(h_out) == H and int(w_out) == W

    base_pose = np.asarray(base_pose, np.float32)
    target_pose = np.asarray(target_pose, np.float32)
    intrinsics = np.asarray(intrinsics, np.float32)
    bg = np.asarray(background_color, np.float32)
    means = np.concatenate([np.asarray(means1, np.float32).reshape(-1, 3),
                            np.asarray(means2, np.float32).reshape(-1, 3)], 0)
    cov = np.concatenate(
        [np.asarray(covariances1, np.float32).reshape(-1, 3, 3),
         np.asarray(covariances2, np.float32).reshape(-1, 3, 3)], 0)
    sh = np.concatenate([np.asarray(sh1, np.float32).reshape(-1, 3, 1),
                         np.asarray(sh2, np.float32).reshape(-1, 3, 1)], 0)
    op = np.concatenate([np.asarray(opacities1, np.float32).reshape(-1),
                         np.asarray(opacities2, np.float32).reshape(-1)], 0)
    assert means.shape[0] == G

    cams = _project(base_pose, target_pose, intrinsics, means, cov, sh, op)

    counts = [int(_band_keep(cams[c], band).sum())
              for c in range(NCAM) for band in range(NBAND)]
    g_dev = 1024 if max(counts) <= 1024 else 2048

    trimat, colsel, carrysel = _shared_consts(g_dev)
    pixbs = [_pix_basis15(band) for band in range(NBAND)]

    in_maps = []
    for core in range(8):
        c, band = divmod(core, NBAND)
        coeff15, dpack, c0, _n = _pack_core(cams[c], band, g_dev, bg)
        in_maps.append({
            "coeff": coeff15, "pixb": pixbs[band], "dpack": dpack,
            "c0col": c0, "trimat": trimat, "colsel": colsel,
            "carrysel": carrysel,
        })

    nc = _get_program(g_dev)

    trace = bool(os.environ.get("BASS_SPLAT_TRACE"))
    kwargs = {}
    if trace:
        kwargs = {"trace": True,
                  "tmpdir": os.environ.get("BASS_SPLAT_TRACE_DIR") or None}
    res = run_bass_kernel_spmd(nc, in_maps, list(range(8)), **kwargs)
    if trace:
        kernel.last_exec_time_ns = res.exec_time_ns
        kernel.last_results = res
    kernel.last_g_dev = g_dev
    kernel.last_counts = counts

    out = np.empty((1, NCAM, 3, H, W), np.float32)
    for core in range(8):
        c, band = divmod(core, NBAND)
        img = res.results[core]["outc"].reshape(3, BAND_ROWS, W)
        out[0, c, :, band * BAND_ROWS:(band + 1) * BAND_ROWS, :] = img
    return out


# revision 14
# speedup vs baseline: 1.0800x; 1.0800x over previous
"""Trainium2 Bass kernel for DecoderSplattingCUDA (Gaussian splatting renderer).

Contract: kernel(**inputs) takes the FULL unsharded inputs (as produced by
setup_inputs) and returns the FULL [1, 2, 3, 64, 64] float32 output.

Sharding: 8 cores = 2 cameras x 4 pixel row-bands (16 rows of 64 px each).
Per-core the host culls the depth-sorted gaussians to those whose alpha can
reach 1/255 inside the band (safe ellipse bound), padded to G_DEV (1024, with
a 2048 fallback program if any core overflows).

Device pipeline per core (gaussians on partitions in blocks of 128, pixels on
the free axis in chunks of 512):
  1. pb = power + ln(op) via K=15 fp16 matmul against a hi/lo-split pixel
     basis (exact to ~1e-3 despite fp16 operands; PSUM accumulates fp32).
  2. alpha = Exp(pb) -> fp16; cutoff alpha<1/255 and clamp at 0.99 via
     is_ge + fused (min, mult) scalar_tensor_tensor (DVE, fp16 2x mode).
  3. L = Ln(1 - alpha) -> fp16 (ACT, scale=-1 bias=1).
  4. Inclusive depth scan S = cumsum(L): upper-triangular fp16 matmul per
     block + per-block carry matmul of previous block totals (PE, fp32 PSUM).
  5. T = Exp(S) -> fp16 = inclusive transmittance.
  6. color = c0 + sum_g T_g * D_g  (Abel summation: D_g = rgb_{g+1} - rgb_g,
     D_last = bg - rgb_last) via [128,3] fp16 matmuls into fp32 PSUM.
"""

import os
import sys

import numpy as np

for _p in ("/opt/trn_rl_repo", "/root/.axon_site/_ro/trn_rl_repo"):
    if os.path.isdir(_p) and _p not in sys.path:
        sys.path.insert(0, _p)
        break

import concourse.bass as bass  # noqa: E402
import concourse.mybir as mybir  # noqa: E402
from concourse.mybir import AluOpType  # noqa: E402
from concourse.tile import TileContext  # noqa: E402
from concourse.bass_utils import run_bass_kernel_spmd  # noqa: E402

# ---------------------------------------------------------------------------
# Workaround: this walrus build only accepts a single sync-wait per
# instruction. Post-pass: hoist all-but-one wait of any multi-wait
# instruction into preceding same-engine NoOps (semantics preserved: the
# engine blocks on each wait in sequence before executing the instruction).
# ---------------------------------------------------------------------------


def _split_multi_waits(nc):
    n = 0
    for f in nc.m.functions:
        for bb in f.blocks:
            new = []
            changed = False
            for ins in bb.instructions:
                si = ins.sync_info
                if si is not None and len(si.on_wait) > 1:
                    changed = True
                    waits = list(si.on_wait)
                    for w in waits[:-1]:
                        n += 1
                        nop = mybir.InstNoOp(name=f"I-wsplit-{n}", ins=[],
                                             outs=[])
                        nop.engine = ins.engine
                        nop.sync_info = mybir.SyncInfo(on_wait=[w],
                                                       on_update=[])
                        new.append(nop)
                    ins.sync_info = mybir.SyncInfo(
                        on_wait=[waits[-1]], on_update=list(si.on_update))
                new.append(ins)
            if changed:
                bb.instructions = new
    return n


# ---------------------------------------------------------------------------
# Problem constants (hardcoded per spec)
# ---------------------------------------------------------------------------
SH_C0 = 0.28209479177387814
NEAR, FAR = 0.1, 1000.0
H = W = 64
G = 2048          # total gaussians (2 context views x 32 x 32)
NCAM = 2
NBAND = 4         # row bands per camera
BAND_ROWS = H // NBAND   # 16
P = BAND_ROWS * W        # 1024 pixels per core
NCHUNK = 2
CH = P // NCHUNK         # 512
F32 = mybir.dt.float32
F16 = mybir.dt.float16
KPOW = 15                # hi/lo-split power matmul contraction size
MASK_CONST = -60000.0    # fp16-representable "minus infinity" for pb
ALPHA_MIN = np.float32(1.0) / np.float32(255.0)
ALPHA_MAX = np.float32(0.99)

_PROGRAMS = {}


def _build_program(g_dev):
    nb = g_dev // 128
    nc = bass.Bass(target_bir_lowering=False)

    coeff = nc.declare_dram_parameter("coeff", [KPOW, g_dev], F16,
                                      isOutput=False)
    pixb = nc.declare_dram_parameter("pixb", [KPOW, P], F16, isOutput=False)
    dpack = nc.declare_dram_parameter("dpack", [128, nb * 3], F16,
                                      isOutput=False)
    c0col = nc.declare_dram_parameter("c0col", [3, 1], F32, isOutput=False)
    trimat = nc.declare_dram_parameter("trimat", [128, 128], F16,
                                       isOutput=False)
    colsel = nc.declare_dram_parameter("colsel", [128, nb * nb], F16,
                                       isOutput=False)
    carrysel = nc.declare_dram_parameter("carrysel", [nb, g_dev], F16,
                                         isOutput=False)
    outc = nc.declare_dram_parameter("outc", [3, P], F32, isOutput=True)

    with TileContext(nc) as tc:
        with (
            tc.tile_pool(name="consts", bufs=1) as consts,
            tc.tile_pool(name="lbuf", bufs=1) as lbuf,
            tc.tile_pool(name="apool", bufs=3) as apool,
            tc.tile_pool(name="a2pool", bufs=3) as a2pool,
            tc.tile_pool(name="mpool", bufs=3) as mpool,
            tc.tile_pool(name="tpool", bufs=3) as tpool,
            tc.tile_pool(name="totsb", bufs=1) as totsb,
            tc.tile_pool(name="outsb", bufs=1) as outsb,
            tc.tile_pool(name="ps_pbs", bufs=2, space="PSUM") as ps_pbs,
            tc.tile_pool(name="ps_tot", bufs=1, space="PSUM") as ps_tot,
            tc.tile_pool(name="ps_col", bufs=1, space="PSUM") as ps_col,
        ):
            # dummy-matmul operand tiles for HAM warm-up bursts (the PE clock
            # gate needs ~3.4us of contiguous matmul activity to open)
            w_lhs = consts.tile([128, 128], F16, tag="w_lhs")
            nc.vector.memset(w_lhs, 0.0)
            w_rhs = consts.tile([128, CH], F16, tag="w_rhs")
            nc.vector.memset(w_rhs, 0.0)

            s_coeff = consts.tile([KPOW, g_dev], F16, tag="coeff")
            nc.sync.dma_start(out=s_coeff, in_=coeff[:])
            s_pixb = consts.tile([KPOW, P], F16, tag="pixb")
            nc.sync.dma_start(out=s_pixb, in_=pixb[:])
            s_dpack = consts.tile([128, nb * 3], F16, tag="dpack")
            nc.sync.dma_start(out=s_dpack, in_=dpack[:])
            s_c0 = consts.tile([3, 1], F32, tag="c0col")
            nc.sync.dma_start(out=s_c0, in_=c0col[:])
            s_tri = consts.tile([128, 128], F16, tag="trimat")
            nc.sync.dma_start(out=s_tri, in_=trimat[:])
            s_colsel = consts.tile([128, nb * nb], F16, tag="colsel")
            nc.sync.dma_start(out=s_colsel, in_=colsel[:])
            s_carry = consts.tile([nb, g_dev], F16, tag="carrysel")
            nc.sync.dma_start(out=s_carry, in_=carrysel[:])

            # big L buffer: [128, nb, P] fp16
            l_all = lbuf.tile([128, nb, P], F16, tag="l_all")
            # psum accumulators alive through the pass
            tot_ps = ps_tot.tile([nb, P], F32, tag="tot")
            col_ps = ps_col.tile([3, P], F32, tag="col")

            # ---- Pass A: alpha + L per gaussian block ----
            # totals matmuls run one block behind so the PE (in-order) never
            # stalls behind a totals MM waiting on the Ln result.
            def emit_totals(b):
                for ch in range(NCHUNK):
                    nc.tensor.matmul(
                        tot_ps[:, ch * CH:(ch + 1) * CH],
                        s_colsel[:, b * nb:(b + 1) * nb],
                        l_all[:, b, ch * CH:(ch + 1) * CH],
                        start=(b == 0),
                        stop=(b == nb - 1),
                    )

            for b in range(nb):
                pb_ps = ps_pbs.tile([128, P], F32, tag="pbs")
                for ch in range(NCHUNK):
                    nc.tensor.matmul(
                        pb_ps[:, ch * CH:(ch + 1) * CH],
                        s_coeff[:, b * 128:(b + 1) * 128],
                        s_pixb[:, ch * CH:(ch + 1) * CH],
                        start=True,
                        stop=True,
                    )
                a_t = apool.tile([128, P], F16, tag="a")
                nc.scalar.activation(
                    a_t, pb_ps, mybir.ActivationFunctionType.Exp,
                )
                # m = (alpha >= 1/255) * 0.99  in {0, 0.99}
                m_t = mpool.tile([128, P], F16, tag="m")
                nc.vector.tensor_scalar(
                    out=m_t, in0=a_t, scalar1=float(ALPHA_MIN),
                    scalar2=float(ALPHA_MAX),
                    op0=AluOpType.is_ge, op1=AluOpType.mult,
                )
                # masked+clamped alpha = min(alpha, m)
                a2_t = a2pool.tile([128, P], F16, tag="a2")
                nc.vector.tensor_tensor(
                    out=a2_t, in0=a_t, in1=m_t, op=AluOpType.min,
                )
                nc.scalar.activation(
                    l_all[:, b, :],
                    a2_t,
                    mybir.ActivationFunctionType.Ln,
                    bias=1.0,
                    scale=-1.0,
                )
                if b > 0:
                    emit_totals(b - 1)
            emit_totals(nb - 1)

            # totals -> SBUF (fp16 cast; |totals| <= ~50 so fp16 rel err ok)
            tot_sb = totsb.tile([nb, P], F16, tag="totsb")
            nc.vector.tensor_copy(tot_sb, tot_ps)

            # keep the PE HAM warm across the pass A -> pass B transition
            # (the totals cast leaves the PE idle long enough to re-throttle)
            for _ in range(9):
                w_ps = ps_pbs.tile([128, CH], F32, tag="pbs")
                nc.tensor.matmul(w_ps, w_lhs, w_rhs, start=True, stop=True)

            # ---- Pass B: scan + carry + T + color ----
            # color matmuls run one block behind (the PE is in-order; a
            # color MM waiting on exp(S_b) must not block scan MMs of b+1)
            def emit_color(b, t_t):
                for ch in range(NCHUNK):
                    nc.tensor.matmul(
                        col_ps[:, ch * CH:(ch + 1) * CH],
                        s_dpack[:, b * 3:(b + 1) * 3],
                        t_t[:, ch * CH:(ch + 1) * CH],
                        start=(b == 0),
                        stop=(b == nb - 1),
                    )

            prev = None
            for b in range(nb):
                s_ps = ps_pbs.tile([128, P], F32, tag="pbs")
                for ch in range(NCHUNK):
                    nc.tensor.matmul(
                        s_ps[:, ch * CH:(ch + 1) * CH],
                        s_tri,
                        l_all[:, b, ch * CH:(ch + 1) * CH],
                        start=True,
                        stop=(b == 0),
                    )
                if b > 0:
                    for ch in range(NCHUNK):
                        nc.tensor.matmul(
                            s_ps[:, ch * CH:(ch + 1) * CH],
                            s_carry[:, b * 128:(b + 1) * 128],
                            tot_sb[:, ch * CH:(ch + 1) * CH],
                            start=False,
                            stop=True,
                        )
                t_t = tpool.tile([128, P], F16, tag="t")
                nc.scalar.activation(
                    t_t, s_ps, mybir.ActivationFunctionType.Exp,
                )
                if prev is not None:
                    emit_color(*prev)
                prev = (b, t_t)
            emit_color(*prev)

            # out = color + c0 (per-partition scalar add folds the Abel
            # constant term into the output copy)
            out_sb = outsb.tile([3, P], F32, tag="outsb")
            nc.vector.tensor_scalar(
                out=out_sb, in0=col_ps, scalar1=s_c0[:, 0:1], scalar2=None,
                op0=AluOpType.add,
            )
            nc.sync.dma_start(out=outc[:], in_=out_sb)

    _split_multi_waits(nc)
    return nc


def _get_program(g_dev):
    if g_dev not in _PROGRAMS:
        _PROGRAMS[g_dev] = _build_program(g_dev)
    return _PROGRAMS[g_dev]


# ---------------------------------------------------------------------------
# Host-side geometry / layout prep
# ---------------------------------------------------------------------------


def _hi_lo(x):
    hi = x.astype(np.float16)
    lo = (x - hi.astype(np.float64)).astype(np.float16)
    return hi, lo


def _project(base_pose, target_pose, intrinsics, means, cov, sh, op):
    """Per-camera projection. Returns per-camera dict of sorted per-gaussian
    quantities (float64 except z which mirrors the reference's fp32)."""
    f32 = np.float32
    inv_base = np.linalg.inv(base_pose.astype(f32))            # [1,4,4]
    extr = np.einsum("bij,bvjk->bvik", inv_base,
                     target_pose.astype(f32)).reshape(NCAM, 4, 4)
    view = np.linalg.inv(extr.astype(f32))
    R = view[:, :3, :3].astype(f32)
    t = view[:, :3, 3].astype(f32)

    K = intrinsics.reshape(NCAM, 3, 3).astype(np.float64)
    cams = []
    for c in range(NCAM):
        p = (means.astype(f32) @ R[c].T.astype(f32) + t[c]).astype(f32)
        z = p[:, 2]                                            # fp32 ref-like
        zc = np.maximum(z, f32(1e-6)).astype(np.float64)
        x = p[:, 0].astype(np.float64)
        y = p[:, 1].astype(np.float64)
        fx, fy = K[c, 0, 0], K[c, 1, 1]
        cx, cy = K[c, 0, 2], K[c, 1, 2]
        u = fx * x / zc + cx
        v = fy * y / zc + cy
        R64 = R[c].astype(np.float64)
        cov_cam = np.einsum("ij,gjk,lk->gil", R64, cov.astype(np.float64),
                            R64)
        w1 = fx / zc
        w2 = -fx * x / zc ** 2
        w3 = fy / zc
        w4 = -fy * y / zc ** 2
        c00, c01, c02 = cov_cam[:, 0, 0], cov_cam[:, 0, 1], cov_cam[:, 0, 2]
        c11, c12, c22 = cov_cam[:, 1, 1], cov_cam[:, 1, 2], cov_cam[:, 2, 2]
        a2d = w1 * w1 * c00 + 2.0 * w1 * w2 * c02 + w2 * w2 * c22 + 0.3
        b2d = (w1 * w3 * c01 + w1 * w4 * c02 + w2 * w3 * c12
               + w2 * w4 * c22)
        d2d = w3 * w3 * c11 + 2.0 * w3 * w4 * c12 + w4 * w4 * c22 + 0.3
        det = a2d * d2d - b2d * b2d
        inv_det = 1.0 / det
        ca = d2d * inv_det
        cb = -b2d * inv_det
        cc = a2d * inv_det
        lam_max_cov = 0.5 * ((a2d + d2d)
                             + np.sqrt((a2d - d2d) ** 2 + 4 * b2d * b2d))
        valid = (z > NEAR) & (z < FAR) & (det > 0) & (op > ALPHA_MIN)
        rgb = np.maximum(SH_C0 * sh[:, :, 0].astype(np.float64) + 0.5, 0.0)
        order = np.argsort(z, kind="stable")
        cams.append(dict(
            u=u[order] - W / 2.0, v=v[order] - H / 2.0,
            ca=ca[order], cb=cb[order], cc=cc[order],
            lam=lam_max_cov[order], valid=valid[order],
            op=np.asarray(op, np.float64)[order], rgb=rgb[order],
        ))
    return cams


def _band_keep(cam, band):
    """Safe cull: keep gaussians whose alpha can reach 1/255 in the band."""
    px_lo, px_hi = 0.5 - W / 2.0, (W - 0.5) - W / 2.0
    py_lo = band * BAND_ROWS + 0.5 - H / 2.0
    py_hi = band * BAND_ROWS + BAND_ROWS - 0.5 - H / 2.0
    dx = np.maximum(np.maximum(px_lo - cam["u"], cam["u"] - px_hi), 0.0)
    dy = np.maximum(np.maximum(py_lo - cam["v"], cam["v"] - py_hi), 0.0)
    # alpha >= 1/255 requires quadform <= 2*(ln op - ln(1/255));
    # quadform >= dist^2 / lam_max(cov2d). 10% + 1px safety inflation.
    budget = 2.0 * (np.log(np.maximum(cam["op"], 1e-12))
                    - np.log(float(ALPHA_MIN)))
    reach2 = np.maximum(budget, 0.0) * cam["lam"]
    reach = np.sqrt(np.maximum(reach2, 0.0)) * 1.05 + 1.0
    return cam["valid"] & (dx * dx + dy * dy <= reach * reach)


def _pack_core(cam, band, g_dev, bg):
    """Build the per-core device inputs for one (camera, band)."""
    keep = _band_keep(cam, band)
    idx = np.nonzero(keep)[0]            # preserves depth order
    n = len(idx)
    assert n <= g_dev
    nb = g_dev // 128

    def sel(a):
        return a[idx]

    u, v = sel(cam["u"]), sel(cam["v"])
    ca, cb, cc = sel(cam["ca"]), sel(cam["cb"]), sel(cam["cc"])
    opk = sel(cam["op"])
    A = -0.5 * (ca * u * u + cc * v * v) - cb * u * v
    const = A + np.log(np.maximum(opk, 1e-12))
    B = ca * u + cb * v
    Cc = cc * v + cb * u
    D = -0.5 * ca
    E = -0.5 * cc
    F = -cb

    def pad(a):
        out = np.zeros(g_dev, np.float64)
        out[:n] = a
        return out

    A6 = [pad(D), pad(E), pad(F), pad(B), pad(Cc)]
    constp = np.full(g_dev, MASK_CONST, np.float64)
    constp[:n] = np.clip(const, MASK_CONST, 0.0)
    # fp16 hi/lo split of the 6 coefficient rows -> 15 rows matching
    # the duplicated pixel basis rows (see _pix_basis15).
    rows = []
    for cf in A6[:3]:                      # quadratic coeffs: hi, hi, lo
        hi, lo = _hi_lo(cf)
        rows += [hi, hi, lo]
    # reorder: we emitted [Dhi,Dhi,Dlo, Ehi,Ehi,Elo, Fhi,Fhi,Flo]
    for cf in A6[3:]:                      # linear coeffs: hi, lo
        hi, lo = _hi_lo(cf)
        rows += [hi, lo]
    khi, klo = _hi_lo(constp)
    rows += [khi, klo]
    coeff15 = np.stack(rows).astype(np.float16)     # [15, g_dev]

    rgb = sel(cam["rgb"])                           # [n, 3]
    if n == 0:
        c0 = bg.astype(np.float64)
        Dr = np.zeros((g_dev, 3), np.float64)
    else:
        c0 = rgb[0]
        rgb_p = np.concatenate(
            [rgb, np.repeat(rgb[-1:], g_dev - n, axis=0)], 0)
        Dr = np.empty((g_dev, 3), np.float64)
        Dr[:-1] = rgb_p[1:] - rgb_p[:-1]
        Dr[-1] = bg.astype(np.float64) - rgb_p[-1]
    dpack = Dr.reshape(nb, 128, 3).transpose(1, 0, 2).reshape(128, nb * 3)
    return (coeff15, dpack.astype(np.float16),
            c0.astype(np.float32).reshape(3, 1), n)


def _shared_consts(g_dev):
    f16 = np.float16
    nb = g_dev // 128
    trimat = (np.arange(128)[:, None] <= np.arange(128)[None, :]).astype(f16)
    colsel = np.zeros((128, nb, nb), f16)
    for b in range(nb):
        colsel[:, b, b] = 1.0
    colsel = colsel.reshape(128, nb * nb)
    carrysel = np.zeros((nb, nb, 128), f16)
    for b in range(nb):
        carrysel[:b, b, :] = 1.0
    carrysel = carrysel.reshape(nb, g_dev)
    return trimat, colsel, carrysel


def _pix_basis15(band):
    px = (np.arange(W, dtype=np.float64) + 0.5) - W / 2.0
    py = (np.arange(BAND_ROWS, dtype=np.float64)
          + band * BAND_ROWS + 0.5) - H / 2.0
    gy, gx = np.meshgrid(py, px, indexing="ij")
    gx = gx.reshape(-1)
    gy = gy.reshape(-1)
    one = np.ones_like(gx)
    q = {}
    for name, val in (("xx", gx * gx), ("yy", gy * gy), ("xy", gx * gy)):
        q[name] = _hi_lo(val)
    # rows match coeff15: [Dhi*xxhi, Dhi*xxlo, Dlo*xxhi] etc.
    rows = [q["xx"][0], q["xx"][1], q["xx"][0],
            q["yy"][0], q["yy"][1], q["yy"][0],
            q["xy"][0], q["xy"][1], q["xy"][0],
            gx, gx, gy, gy, one, one]
    return np.stack([np.asarray(r, np.float64) for r in rows]).astype(np.float16)


def kernel(base_pose, target_pose, intrinsics, means1, covariances1, sh1,
           opacities1, means2, covariances2, sh2, opacities2,
           background_color, h_out, w_out):
    assert int(h_out) == H and int(w_out) == W

    base_pose = np.asarray(base_pose, np.float32)
    target_pose = np.asarray(target_pose, np.float32)
    intrinsics = np.asarray(intrinsics, np.float32)
    bg = np.asarray(background_color, np.float32)
    means = np.concatenate([np.asarray(means1, np.float32).reshape(-1, 3),
                            np.asarray(means2, np.float32).reshape(-1, 3)], 0)
    cov = np.concatenate(
        [np.asarray(covariances1, np.float32).reshape(-1, 3, 3),
         np.asarray(covariances2, np.float32).reshape(-1, 3, 3)], 0)
    sh = np.concatenate([np.asarray(sh1, np.float32).reshape(-1, 3, 1),
                         np.asarray(sh2, np.float32).reshape(-1, 3, 1)], 0)
    op = np.concatenate([np.asarray(opacities1, np.float32).reshape(-1),
                         np.asarray(opacities2, np.float32).reshape(-1)], 0)
    assert means.shape[0] == G

    cams = _project(base_pose, target_pose, intrinsics, means, cov, sh, op)

    counts = [int(_band_keep(cams[c], band).sum())
              for c in range(NCAM) for band in range(NBAND)]
    g_dev = 1024 if max(counts) <= 1024 else 2048

    trimat, colsel, carrysel = _shared_consts(g_dev)
    pixbs = [_pix_basis15(band) for band in range(NBAND)]

    in_maps = []
    for core in range(8):
        c, band = divmod(core, NBAND)
        coeff15, dpack, c0, _n = _pack_core(cams[c], band, g_dev, bg)
        in_maps.append({
            "coeff": coeff15, "pixb": pixbs[band], "dpack": dpack,
            "c0col": c0, "trimat": trimat, "colsel": colsel,
            "carrysel": carrysel,
        })

    nc = _get_program(g_dev)

    trace = bool(os.environ.get("BASS_SPLAT_TRACE"))
    kwargs = {}
    if trace:
        kwargs = {"trace": True,
                  "tmpdir": os.environ.get("BASS_SPLAT_TRACE_DIR") or None}
    res = run_bass_kernel_spmd(nc, in_maps, list(range(8)), **kwargs)
    if trace:
        kernel.last_exec_time_ns = res.exec_time_ns
        kernel.last_results = res
    kernel.last_g_dev = g_dev
    kernel.last_counts = counts

    out = np.empty((1, NCAM, 3, H, W), np.float32)
    for core in range(8):
        c, band = divmod(core, NBAND)
        img = res.results[core]["outc"].reshape(3, BAND_ROWS, W)
        out[0, c, :, band * BAND_ROWS:(band + 1) * BAND_ROWS, :] = img
    return out
